# revision 3
# baseline (speedup 1.0000x reference)
"""AttentionPooling (segment softmax-weighted scatter) Trainium2 Bass kernel.

Strategy (8 NeuronCores, SPMD):
  - Shard by SEGMENT BLOCKS: core c owns segments [c*128, (c+1)*128) and all
    nodes whose (sorted) batch id falls in that range. No cross-core reduction
    is needed: each segment lives entirely on one core. Host pads each core's
    node count to a common T_pad tiles of 128 so the compiled program is
    identical across cores.
  - Scores are computed without max-subtraction: p = exp(s + b2) directly.
    Scores are bounded (|tanh|<=1, |W2| small) so raw exp is safe in fp32,
    and the reference's +1e-8 epsilon is negligible relative to seg sums.
  - Phase A (per core): s = tanh(x @ W1 + b1) @ W2 using a host-fed
    transposed copy of x (hidden on partitions), nodes on the free dim.
  - Phase B: one-hot weighted scatter matmul. S[i, m] = (m == c_i) * p_i
    built by a single dual-op tensor_scalar; out[seg, :] += S^T @ [x | 1]
    accumulated in PSUM over all tiles.
  - Host divides wx by the sum column and concatenates core outputs.

Inputs are fed in bf16 (x twice: natural and transposed layouts); all
accumulation is fp32 in PSUM.
"""

import math
from functools import lru_cache

import ml_dtypes
import numpy as np

import concourse.mybir as mybir
import concourse.tile as tile
from concourse import bacc
from concourse.bass_utils import run_bass_kernel_spmd
from concourse.masks import make_identity

P = 128          # partitions / tile rows
HID = 256        # hidden dim
H2 = 128         # MLP inner dim
NSEG = 1024      # segments (batch size)
NCORES = 8
F = 512          # phase-A chunk (nodes per score chunk)
FB = F // P      # tiles per chunk

BF16 = mybir.dt.bfloat16
F32 = mybir.dt.float32
NPBF16 = ml_dtypes.bfloat16


def build_kernel(chunks: int):
    """Build the per-core SPMD program for `chunks` phase-A chunks
    (T = chunks*FB tiles of 128 nodes, N_pad = T*128 nodes per core)."""
    assert chunks <= P
    T = chunks * FB
    n_pad = T * P

    nc = bacc.Bacc("TRN2")
    x_in = nc.dram_tensor("x", [n_pad, HID], BF16, kind="ExternalInput")
    xT_in = nc.dram_tensor("xT", [HID, n_pad], BF16, kind="ExternalInput")
    c_in = nc.dram_tensor("c", [P, T], F32, kind="ExternalInput")
    w1_in = nc.dram_tensor("w1", [HID, H2], BF16, kind="ExternalInput")
    w2_in = nc.dram_tensor("w2", [H2, 1], BF16, kind="ExternalInput")
    b1_in = nc.dram_tensor("b1", [H2, 1], F32, kind="ExternalInput")
    b2_in = nc.dram_tensor("b2", [P, 1], F32, kind="ExternalInput")
    iota_in = nc.dram_tensor("iota", [P, P], F32, kind="ExternalInput")
    out_t = nc.dram_tensor("out", [P, HID + 1], F32, kind="ExternalOutput")
    scores_d = nc.dram_tensor("scores", [chunks, F], F32)

    with tile.TileContext(nc) as tc:
        with (
            tc.tile_pool(name="const", bufs=1) as cpool,
            tc.tile_pool(name="xT", bufs=3) as xT_pool,
            tc.tile_pool(name="th", bufs=3) as th_pool,
            tc.tile_pool(name="bounce", bufs=3) as b_pool,
            tc.tile_pool(name="x4", bufs=3) as x4_pool,
            tc.tile_pool(name="S", bufs=4) as s_pool,
            tc.tile_pool(name="ph", bufs=2, space="PSUM") as ph_pool,
            tc.tile_pool(name="ps", bufs=2, space="PSUM") as ps_pool,
            tc.tile_pool(name="pT", bufs=1, space="PSUM") as pT_pool,
            tc.tile_pool(name="po", bufs=1, space="PSUM") as po_pool,
            tc.tile_pool(name="po2", bufs=1, space="PSUM") as po2_pool,
        ):
            # ---- constants ----
            w1a = cpool.tile([P, H2], BF16, tag="w1a")
            w1b = cpool.tile([P, H2], BF16, tag="w1b")
            w2t = cpool.tile([H2, 1], BF16, tag="w2t")
            b1t = cpool.tile([H2, 1], F32, tag="b1t")
            b2t = cpool.tile([P, 1], F32, tag="b2t")
            iota_t = cpool.tile([P, P], F32, tag="iota")
            ident = cpool.tile([P, P], F32, tag="ident")
            ones_c = cpool.tile([P, 1], BF16, tag="ones")
            c_cols = cpool.tile([P, T], F32, tag="ccols")
            p_cols = cpool.tile([P, T], F32, tag="pcols")
            scores_sb = cpool.tile([chunks, F], F32, tag="ssb")
            out_sb = cpool.tile([P, HID + 1], F32, tag="osb")

            nc.sync.dma_start(out=w1a[:], in_=w1_in[0:P, :])
            nc.sync.dma_start(out=w1b[:], in_=w1_in[P:HID, :])
            nc.sync.dma_start(out=w2t[:], in_=w2_in[:])
            nc.sync.dma_start(out=b1t[:], in_=b1_in[:])
            nc.sync.dma_start(out=b2t[:], in_=b2_in[:])
            nc.sync.dma_start(out=iota_t[:], in_=iota_in[:])
            nc.sync.dma_start(out=c_cols[:], in_=c_in[:])
            nc.vector.memset(ones_c[:], 1.0)
            make_identity(nc, ident[:])

            # ---- phase A: scores ----
            for g in range(chunks):
                xTa = xT_pool.tile([P, F], BF16, tag="xTa")
                xTb = xT_pool.tile([P, F], BF16, tag="xTb")
                nc.sync.dma_start(out=xTa[:], in_=xT_in[0:P, g * F:(g + 1) * F])
                nc.sync.dma_start(out=xTb[:], in_=xT_in[P:HID, g * F:(g + 1) * F])
                ph = ph_pool.tile([P, F], F32)
                nc.tensor.matmul(out=ph[:], lhsT=w1a[:], rhs=xTa[:],
                                 start=True, stop=False)
                nc.tensor.matmul(out=ph[:], lhsT=w1b[:], rhs=xTb[:],
                                 start=False, stop=True)
                th = th_pool.tile([P, F], BF16)
                nc.scalar.activation(out=th[:], in_=ph[:],
                                     func=mybir.ActivationFunctionType.Tanh,
                                     bias=b1t[:], scale=1.0)
                ps = ps_pool.tile([1, F], F32)
                nc.tensor.matmul(out=ps[:], lhsT=w2t[:], rhs=th[:],
                                 start=True, stop=True)
                bounce = b_pool.tile([1, F], F32)
                if g % 2 == 0:
                    nc.scalar.copy(out=bounce[:], in_=ps[:])
                else:
                    nc.vector.tensor_copy(out=bounce[:], in_=ps[:])
                nc.sync.dma_start(out=scores_d[g:g + 1, :], in_=bounce[:])

            # ---- scores -> p columns ----
            nc.sync.dma_start(out=scores_sb[:], in_=scores_d[:])
            pT = pT_pool.tile([P, T], F32)
            for fb in range(FB):
                nc.tensor.transpose(
                    out=pT[:, fb * chunks:(fb + 1) * chunks],
                    in_=scores_sb[:, fb * P:(fb + 1) * P],
                    identity=ident[:chunks, :chunks])
            nc.scalar.activation(out=p_cols[:], in_=pT[:],
                                 func=mybir.ActivationFunctionType.Exp,
                                 bias=b2t[:], scale=1.0)

            # ---- phase B: weighted scatter ----
            x_r = x_in[:].rearrange("(n f p) d -> f n p d", f=FB, p=P)
            po = po_pool.tile([P, HID], F32)
            po2 = po2_pool.tile([P, 1], F32)
            for j in range(T):
                fb, n = j // chunks, j % chunks
                i4 = n % 4
                if i4 == 0:
                    x4 = x4_pool.tile([P, 4 * HID], BF16)
                    nc.sync.dma_start(
                        out=x4[:].rearrange("p (n d) -> p n d", d=HID),
                        in_=x_r[fb, n:n + 4].rearrange("n p d -> p n d"))
                S = s_pool.tile([P, P], BF16)
                nc.vector.tensor_scalar(
                    out=S[:], in0=iota_t[:],
                    scalar1=c_cols[:, j:j + 1], scalar2=p_cols[:, j:j + 1],
                    op0=mybir.AluOpType.is_equal, op1=mybir.AluOpType.mult)
                nc.tensor.matmul(out=po[:], lhsT=S[:],
                                 rhs=x4[:, i4 * HID:(i4 + 1) * HID],
                                 start=(j == 0), stop=(j == T - 1),
                                 skip_group_check=True)
                nc.tensor.matmul(out=po2[:], lhsT=S[:],
                                 rhs=ones_c[:],
                                 start=(j == 0), stop=(j == T - 1),
                                 skip_group_check=True)
            nc.vector.tensor_copy(out=out_sb[:, 0:HID], in_=po[:])
            nc.vector.tensor_copy(out=out_sb[:, HID:HID + 1], in_=po2[:])
            nc.sync.dma_start(out=out_t[:], in_=out_sb[:])

    nc.finalize()
    return nc


@lru_cache(maxsize=4)
def _compiled(chunks: int):
    return build_kernel(chunks)


def _prep_inputs(x, batch, W1, b1, W2, b2):
    """Shard by segment blocks; build padded per-core arrays."""
    x = np.asarray(x, dtype=np.float32)
    batch = np.asarray(batch).astype(np.int64)
    bounds = np.searchsorted(batch, np.arange(0, NSEG + 1, P))
    counts = np.diff(bounds)
    maxn = int(counts.max())
    chunks = ((maxn + F - 1) // F + 3) // 4 * 4  # multiple of 4 for DMA batching
    chunks = max(chunks, 4)
    assert chunks <= P, f"core node count {maxn} exceeds capacity"
    T = chunks * FB
    n_pad = T * P

    # tile order used on device: column j <-> tile t = 4*(j%chunks) + j//chunks
    js = np.arange(T)
    t_order = 4 * (js % chunks) + js // chunks

    x_dev = np.zeros((NCORES, n_pad, HID), dtype=NPBF16)
    xT_dev = np.zeros((NCORES, HID, n_pad), dtype=NPBF16)
    c_dev = np.empty((NCORES, P, T), dtype=np.float32)
    for core in range(NCORES):
        s, e = int(bounds[core]), int(bounds[core + 1])
        n = e - s
        xs = x[s:e].astype(NPBF16)
        x_dev[core, :n] = xs
        xT_dev[core, :, :n] = np.ascontiguousarray(xs.T)
        c_all = np.full(n_pad, -1000.0, dtype=np.float32)
        c_all[:n] = (batch[s:e] - core * P).astype(np.float32)
        c_dev[core] = c_all.reshape(T, P)[t_order].T

    w1 = W1.astype(NPBF16)
    w2 = W2.astype(NPBF16).reshape(H2, 1)
    b1c = b1.astype(np.float32).reshape(H2, 1)
    b2c = np.full((P, 1), np.float32(np.asarray(b2).reshape(-1)[0]))
    iota = np.broadcast_to(np.arange(P, dtype=np.float32), (P, P)).copy()

    in_maps = []
    for core in range(NCORES):
        in_maps.append({
            "x": x_dev[core], "xT": xT_dev[core], "c": c_dev[core],
            "w1": w1, "w2": w2, "b1": b1c, "b2": b2c, "iota": iota,
        })
    return chunks, in_maps


def kernel(x, batch, W1, b1, W2, b2):
    chunks, in_maps = _prep_inputs(x, batch, W1, b1, W2, b2)
    nc = _compiled(chunks)
    res = run_bass_kernel_spmd(nc, in_maps, core_ids=list(range(NCORES)))
    parts = np.stack([res.results[i]["out"] for i in range(NCORES)])  # [8,128,257]
    wx = parts[:, :, :HID].reshape(NSEG, HID)
    ssum = parts[:, :, HID].reshape(NSEG, 1)
    out = np.divide(wx, ssum, out=np.zeros_like(wx), where=ssum != 0)
    return out.astype(np.float32)


# revision 28
# speedup vs baseline: 337.0235x; 337.0235x over previous
"""AttentionPooling (segment softmax-weighted scatter) Trainium2 Bass kernel.

Strategy (8 NeuronCores, SPMD):
  - Shard by SEGMENT BLOCKS: core c owns segments [c*128, (c+1)*128) and all
    nodes whose (sorted) batch id falls in that range. No cross-core reduction
    is needed: each segment lives entirely on one core. Host pads each core's
    node count to a common T tiles of 128 so the compiled program is identical
    across cores.
  - Scores are computed without max-subtraction: p = exp(s + b2) directly.
    Scores are bounded (|tanh|<=1, |W2| small) so raw exp is safe in fp32,
    and the reference's +1e-8 epsilon is negligible relative to seg sums.
  - Phase A: s = tanh(x @ W1 + b1) @ W2 from a host-fed transposed copy of x
    (hidden on partitions, nodes on the free dim).
  - Phase B: one-hot weighted scatter. S[i, m] = (m == c_i) * p_i built by one
    dual-op tensor_scalar; out[seg, :] += S^T @ x and sums += S^T @ 1
    accumulate in PSUM over all tiles (separate banks).
  - A and B are pipelined in NGROUPS groups: group q's scatter overlaps group
    q+1's scores.
  - Host divides wx by the sum column and concatenates core outputs.

x is fed twice in bf16 (swizzled natural layout + transposed layout), both
arranged so every DMA reads >=2KB contiguous per partition. fp32 accumulation
in PSUM throughout.
"""

from functools import lru_cache

import ml_dtypes
import numpy as np

import concourse.mybir as mybir
import concourse.tile as tile
from concourse import bacc
from concourse.masks import make_identity

P = 128          # partitions / tile rows
HID = 256        # hidden dim
H2 = 128         # MLP inner dim
NSEG = 1024      # segments (batch size)
NCORES = 8
F = 512          # phase-A chunk (nodes per score chunk)
FB = F // P      # tiles per chunk
NGROUPS = 2      # A/B pipeline groups

BF16 = mybir.dt.bfloat16
F32 = mybir.dt.float32
NPBF16 = ml_dtypes.bfloat16


SB = 32  # chunks per sub-bridge


def tile_order(chunks: int) -> np.ndarray:
    """Device iteration order: tile index t for each phase-B step j."""
    G = chunks // NGROUPS
    sb = min(SB, G)
    j = np.arange(chunks * FB)
    q, r = j // (FB * G), j % (FB * G)
    k, s = r // (FB * sb), r % (FB * sb)
    fb, nl = s // sb, s % sb
    return FB * (q * G + k * sb + nl) + fb


def build_kernel(chunks: int):
    assert chunks % (4 * NGROUPS) == 0 and chunks <= P
    G = chunks // NGROUPS          # chunks per group
    T = chunks * FB                # node tiles per core
    n_pad = T * P
    XTP = 8 if G % 8 == 0 else 4
    sb = min(SB, G)
    assert G % sb == 0 and G % XTP == 0

    nc = bacc.Bacc("TRN2")
    # x is host-swizzled: x[j8, p, i*HID:(i+1)*HID] = node-tile t(8*j8+i) lane p
    x_in = nc.dram_tensor("x", [T // 8, P, 8 * HID], BF16, kind="ExternalInput")
    xT_in = nc.dram_tensor("xT", [HID, n_pad], BF16, kind="ExternalInput")
    c_in = nc.dram_tensor("c", [P, T], F32, kind="ExternalInput")
    w1_in = nc.dram_tensor("w1", [HID, H2], BF16, kind="ExternalInput")
    w2_in = nc.dram_tensor("w2", [H2, 1], BF16, kind="ExternalInput")
    b1_in = nc.dram_tensor("b1", [H2, 1], F32, kind="ExternalInput")
    b2_in = nc.dram_tensor("b2", [P, 1], F32, kind="ExternalInput")
    iota_in = nc.dram_tensor("iota", [P, P], BF16, kind="ExternalInput")
    out_t = nc.dram_tensor("out", [P, HID], F32, kind="ExternalOutput")
    scores_d = nc.dram_tensor("scores", [chunks, F], F32, kind="ExternalOutput")

    with tile.TileContext(nc) as tc:
        with (
            tc.tile_pool(name="const", bufs=1) as cpool,
            tc.tile_pool(name="xT", bufs=5) as xT_pool,
            tc.tile_pool(name="th", bufs=4) as th_pool,
            tc.tile_pool(name="bounce", bufs=4) as b_pool,
            tc.tile_pool(name="ssb", bufs=3) as ssb_pool,
            tc.tile_pool(name="x4", bufs=8) as x4_pool,
            tc.tile_pool(name="S", bufs=6) as s_pool,
            tc.tile_pool(name="ph", bufs=3, space="PSUM") as ph_pool,
            tc.tile_pool(name="ps", bufs=2, space="PSUM") as ps_pool,
            tc.tile_pool(name="pT", bufs=1, space="PSUM") as pT_pool,
            tc.tile_pool(name="po", bufs=1, space="PSUM") as po_pool,
        ):
            # ---- constants ----
            w1a = cpool.tile([P, H2], BF16, tag="w1a")
            w1b = cpool.tile([P, H2], BF16, tag="w1b")
            w2t = cpool.tile([H2, 1], BF16, tag="w2t")
            b1t = cpool.tile([H2, 1], F32, tag="b1t")
            b2t = cpool.tile([P, 1], F32, tag="b2t")
            iota_t = cpool.tile([P, P], BF16, tag="iota")
            ident = cpool.tile([P, P], F32, tag="ident")
            c_cols = cpool.tile([P, T], F32, tag="ccols")
            p_cols = cpool.tile([P, T], F32, tag="pcols")
            out_sb = cpool.tile([P, HID], F32, tag="osb")

            nc.gpsimd.dma_start(out=w1a[:], in_=w1_in[0:P, :])
            nc.gpsimd.dma_start(out=w1b[:], in_=w1_in[P:HID, :])
            nc.gpsimd.dma_start(out=w2t[:], in_=w2_in[:])
            nc.gpsimd.dma_start(out=b1t[:], in_=b1_in[:])
            nc.gpsimd.dma_start(out=b2t[:], in_=b2_in[:])
            nc.gpsimd.dma_start(out=iota_t[:], in_=iota_in[:])
            nc.gpsimd.dma_start(out=c_cols[:], in_=c_in[:])
            make_identity(nc, ident[:])

            po = po_pool.tile([P, HID], F32)

            def phase_a(q):
                for g in range(q * G, (q + 1) * G):
                    phase_a_chunk(g)
                    if (g + 1) % sb == 0:
                        sub_bridge(g)

            xT_holder = [None, None]
            bounce_holder = [None]

            def phase_a_chunk(g):
                    if g % XTP == 0:
                        xT_holder[0] = xT_pool.tile([P, XTP * F], BF16, tag="xTa", name="xTa")
                        xT_holder[1] = xT_pool.tile([P, XTP * F], BF16, tag="xTb", name="xTb")
                        nc.sync.dma_start(
                            out=xT_holder[0][:], in_=xT_in[0:P, g * F:(g + XTP) * F])
                        nc.sync.dma_start(
                            out=xT_holder[1][:], in_=xT_in[P:HID, g * F:(g + XTP) * F])
                    xTa, xTb = xT_holder
                    g4 = (g % XTP) * F
                    ph = ph_pool.tile([P, F], F32)
                    nc.tensor.matmul(out=ph[:], lhsT=w1a[:],
                                     rhs=xTa[:, g4:g4 + F],
                                     start=True, stop=False)
                    nc.tensor.matmul(out=ph[:], lhsT=w1b[:],
                                     rhs=xTb[:, g4:g4 + F],
                                     start=False, stop=True)
                    th = th_pool.tile([P, F], BF16)
                    nc.scalar.activation(out=th[:], in_=ph[:],
                                         func=mybir.ActivationFunctionType.Tanh,
                                         bias=b1t[:], scale=1.0)
                    ps = ps_pool.tile([1, F], F32)
                    nc.tensor.matmul(out=ps[:], lhsT=w2t[:], rhs=th[:],
                                     start=True, stop=True)
                    if g % 4 == 0:
                        bounce_holder[0] = b_pool.tile([1, 4 * F], F32, tag="bounce", name="bounce")
                    bounce = bounce_holder[0]
                    gb = (g % 4) * F
                    nc.vector.tensor_copy(out=bounce[:, gb:gb + F], in_=ps[:])
                    if g % 4 == 3:
                        dst = (scores_d[:].rearrange("c f -> (c f)")
                               [(g - 3) * F:(g + 1) * F][None, :])
                        nc.gpsimd.dma_start(out=dst, in_=bounce[:])

            def sub_bridge(g_hi):
                """Transpose+exp scores of chunks [g_hi-sb+1 .. g_hi]."""
                g_lo = g_hi - sb + 1
                ssb = ssb_pool.tile([sb, F], F32)
                nc.gpsimd.dma_start(out=ssb[:], in_=scores_d[g_lo:g_hi + 1, :])
                pT = pT_pool.tile([P, FB * sb], F32)
                for fb in range(FB):
                    nc.tensor.transpose(
                        out=pT[:, fb * sb:(fb + 1) * sb],
                        in_=ssb[:, fb * P:(fb + 1) * P],
                        identity=ident[:sb, :sb])
                nc.scalar.activation(
                    out=p_cols[:, g_lo * FB:(g_hi + 1) * FB], in_=pT[:],
                    func=mybir.ActivationFunctionType.Exp, bias=b2t[:], scale=1.0)

            x4_holder = [None]

            def phase_b_tiles(js):
                for j in js:
                    i8 = j % 8
                    if i8 == 0:
                        x4_holder[0] = x4_pool.tile([P, 8 * HID], BF16, tag="x4", name="x4")
                        nc.sync.dma_start(out=x4_holder[0][:], in_=x_in[j // 8])
                    x4 = x4_holder[0]
                    S = s_pool.tile([P, P], BF16, tag="S")
                    nc.vector.tensor_scalar(
                        out=S[:], in0=iota_t[:],
                        scalar1=c_cols[:, j:j + 1], scalar2=p_cols[:, j:j + 1],
                        op0=mybir.AluOpType.is_equal, op1=mybir.AluOpType.mult)
                    nc.tensor.matmul(out=po[:], lhsT=S[:],
                                     rhs=x4[:, i8 * HID:(i8 + 1) * HID],
                                     start=(j == 0), stop=(j == T - 1),
                                     skip_group_check=True)

            def interleaved(q):
                """Emit phase_a(q) chunks interleaved with phase_b(q-1) tiles."""
                jb = (q - 1) * FB * G
                for gi, g in enumerate(range(q * G, (q + 1) * G)):
                    phase_a_chunk(g)
                    if (g + 1) % sb == 0:
                        sub_bridge(g)
                    if gi % 2 == 0:
                        phase_b_tiles(range(jb + gi * FB, jb + (gi + 2) * FB))

            phase_a(0)
            for q in range(1, NGROUPS):
                interleaved(q)
            phase_b_tiles(range((NGROUPS - 1) * FB * G, NGROUPS * FB * G))

            nc.vector.tensor_copy(out=out_sb[:], in_=po[:])
            nc.gpsimd.dma_start(out=out_t[:], in_=out_sb[:])

    nc.finalize()
    return nc


@lru_cache(maxsize=4)
def _compiled(chunks: int):
    return build_kernel(chunks)


@lru_cache(maxsize=4)
def _runner(chunks: int):
    """Persistent jitted shard_map over the 8 cores (compiles once)."""
    import jax
    from concourse import bass2jax
    from jax.sharding import Mesh, PartitionSpec
    from jax.experimental.shard_map import shard_map

    nc = _compiled(chunks)
    bass2jax.install_neuronx_cc_hook()
    partition_name = nc.partition_id_tensor.name if nc.partition_id_tensor else None
    in_names, out_names, out_avals, zero_outs = [], [], [], []
    for alloc in nc.m.functions[0].allocations:
        if not isinstance(alloc, mybir.MemoryLocationSet):
            continue
        name = alloc.memorylocations[0].name
        if alloc.kind == "ExternalInput":
            if name != partition_name:
                in_names.append(name)
        elif alloc.kind == "ExternalOutput":
            out_names.append(name)
            shape = tuple(alloc.tensor_shape)
            dtype = mybir.dt.np(alloc.dtype)
            out_avals.append(jax.core.ShapedArray(shape, dtype))
            zero_outs.append(np.zeros(shape, dtype))
    n_params = len(in_names)
    all_in_names = list(in_names) + list(out_names)
    if partition_name is not None:
        all_in_names.append(partition_name)

    def _body(*args):
        operands = list(args)
        if partition_name is not None:
            operands.append(bass2jax.partition_id_tensor())
        outs = bass2jax._bass_exec_p.bind(
            *operands,
            out_avals=tuple(out_avals),
            in_names=tuple(all_in_names),
            out_names=tuple(out_names),
            lowering_input_output_aliases=(),
            sim_require_finite=True,
            sim_require_nnan=True,
            nc=nc,
        )
        return tuple(outs)

    devices = jax.devices()[:NCORES]
    assert len(devices) >= NCORES
    mesh = Mesh(np.asarray(devices), ("core",))
    in_specs = (PartitionSpec("core"),) * (n_params + len(out_names))
    out_specs = (PartitionSpec("core"),) * len(out_names)
    sharded = jax.jit(
        shard_map(_body, mesh=mesh, in_specs=in_specs, out_specs=out_specs,
                  check_rep=False),
        keep_unused=True,
    )
    concat_zeros = [
        np.zeros((NCORES * z.shape[0], *z.shape[1:]), z.dtype) for z in zero_outs
    ]

    def run(in_maps):
        concat_in = [
            np.concatenate([np.asarray(in_maps[c][n]) for c in range(NCORES)],
                           axis=0)
            for n in in_names
        ]
        out = sharded(*concat_in, *concat_zeros)
        return {
            name: np.asarray(out[i]).reshape(NCORES, *out_avals[i].shape)
            for i, name in enumerate(out_names)
        }

    return run


def _prep_inputs(x, batch, W1, b1, W2, b2):
    """Shard by segment blocks; build padded per-core arrays."""
    x = np.asarray(x, dtype=np.float32)
    batch = np.asarray(batch).astype(np.int64)
    bounds = np.searchsorted(batch, np.arange(0, NSEG + 1, P))
    counts = np.diff(bounds)
    maxn = int(counts.max())
    chunks = -(-maxn // F)
    step = 4 * NGROUPS
    chunks = -(-chunks // step) * step  # DMA/pipeline alignment
    assert chunks <= P, f"core node count {maxn} exceeds capacity"
    T = chunks * FB
    n_pad = T * P
    t_order = tile_order(chunks)

    x_dev = np.zeros((NCORES, T // 8, P, 8 * HID), dtype=NPBF16)
    xT_dev = np.zeros((NCORES, HID, n_pad), dtype=NPBF16)
    c_dev = np.empty((NCORES, P, T), dtype=np.float32)
    for core in range(NCORES):
        s, e = int(bounds[core]), int(bounds[core + 1])
        n = e - s
        xs = x[s:e].astype(NPBF16)
        x_pad = np.zeros((n_pad, HID), dtype=NPBF16)
        x_pad[:n] = xs
        x_dev[core] = (x_pad.reshape(T, P, HID)[t_order]
                       .reshape(T // 8, 8, P, HID)
                       .transpose(0, 2, 1, 3)
                       .reshape(T // 8, P, 8 * HID))
        xT_dev[core, :, :n] = np.ascontiguousarray(xs.T)
        c_all = np.full(n_pad, -1000.0, dtype=np.float32)
        c_all[:n] = (batch[s:e] - core * P).astype(np.float32)
        c_dev[core] = c_all.reshape(T, P)[t_order].T

    w1 = np.asarray(W1, dtype=np.float32).astype(NPBF16)
    w2 = np.asarray(W2, dtype=np.float32).astype(NPBF16).reshape(H2, 1)
    b1c = np.asarray(b1, dtype=np.float32).reshape(H2, 1)
    b2c = np.full((P, 1), np.float32(np.asarray(b2).reshape(-1)[0]))
    iota = np.broadcast_to(np.arange(P, dtype=np.float32), (P, P)).astype(NPBF16)

    in_maps = []
    for core in range(NCORES):
        in_maps.append({
            "x": x_dev[core], "xT": xT_dev[core], "c": c_dev[core],
            "w1": w1, "w2": w2, "b1": b1c, "b2": b2c, "iota": iota,
        })
    return chunks, in_maps


def _host_ssum(scores, batch, b2):
    """Per-segment sum of p = exp(score + b2), from exported per-core scores.

    scores[core] is [chunks, F] over that core's padded node stream; entry
    (g, f) is node g*F + f of the core's stream. Padded nodes are excluded by
    counting only the first n_c real nodes."""
    batch = np.asarray(batch).astype(np.int64)
    bounds = np.searchsorted(batch, np.arange(0, NSEG + 1, P))
    b2v = np.float32(np.asarray(b2, dtype=np.float32).reshape(-1)[0])
    ssum = np.zeros((NSEG, 1), dtype=np.float32)
    for core in range(NCORES):
        s, e = int(bounds[core]), int(bounds[core + 1])
        n = e - s
        p = np.exp(scores[core].reshape(-1)[:n].astype(np.float32) + b2v)
        seg = batch[s:e]
        ssum[:, 0] += np.bincount(seg, weights=p, minlength=NSEG).astype(np.float32)
    return ssum


def kernel(x, batch, W1, b1, W2, b2):
    batch = np.asarray(batch)
    chunks, in_maps = _prep_inputs(x, batch, W1, b1, W2, b2)
    try:
        res = _runner(chunks)(in_maps)
        wx = res["out"].reshape(NSEG, HID)
        scores = res["scores"]
    except Exception:
        # fall back to the stock SPMD driver (recompiles per call)
        from concourse.bass_utils import run_bass_kernel_spmd
        r = run_bass_kernel_spmd(_compiled(chunks), in_maps,
                                 core_ids=list(range(NCORES)))
        wx = np.concatenate([r.results[i]["out"] for i in range(NCORES)], axis=0)
        scores = np.stack([r.results[i]["scores"] for i in range(NCORES)])
    ssum = _host_ssum(scores, batch, b2)
    out = np.divide(wx, ssum, out=np.zeros_like(wx), where=ssum != 0)
    return out.astype(np.float32)

